# revision 15
# baseline (speedup 1.0000x reference)
"""GraphSAGE (2-layer + decoder) on 8 TRN2 NeuronCores.

Sharding: nodes partitioned across 8 cores (dst-partitioned edges).
Layer 1 feeds on a host-gathered, edge-ordered copy of x (sequential DMA);
relu(h1) shards are AllGathered in two pieces (bf16), and layer 2 gathers
source rows on-device with indirect DMA. Segment-mean is computed as one-hot
matmuls accumulating into per-window PSUM; dense lin_l/lin_r/bias terms and
the decoder are fused per 128-node block. Decoder weights are pre-composed
with conv2 weights on the host so dx needs no extra transpose.

Pipelining: each core's nodes are split into an early slice A (first A_WIN
windows) and the rest B. Sequence: L1(A) -> AllGather(A) -> L1(B) with
layer-2 gathers/aggregation for A-sourced edges overlapping it -> AG(B) ->
layer-2 B-sourced aggregation (seeded with the buffered A partials) +
epilogues. This keeps the serialized indirect-DMA stream (the bottleneck)
running as early as possible.
"""
import os
import sys

sys.path.insert(0, '/opt/trn_rl_repo')

import numpy as np
import ml_dtypes

import concourse.bass as bass
import concourse.bacc as bacc
import concourse.mybir as mybir
import concourse.tile as tile
from concourse.masks import make_identity

bf16 = ml_dtypes.bfloat16
dt = mybir.dt

C = 8           # cores
F = 128         # features/hidden
WIN = 256       # dst nodes per aggregation window (one-hot width)
P = 128         # partitions / chunk size


def _schedule(src, dst, n_nodes):
    """Shared (SPMD-uniform) schedule with (window, source-half) cells."""
    NC_ = n_nodes // C
    n_win = (NC_ + WIN - 1) // WIN
    NPAD = n_win * WIN
    a_win = max(1, n_win // 5)
    A_LOC = a_win * WIN                      # early-slice nodes per core
    deg = np.bincount(dst, minlength=n_nodes)
    invdeg = (1.0 / np.maximum(deg, 1)).astype(np.float32)

    cores = []
    cntA = np.zeros((C, n_win), np.int64)
    cntB = np.zeros((C, n_win), np.int64)
    for c in range(C):
        m = (dst >= c * NC_) & (dst < (c + 1) * NC_)
        ed = (dst[m] - c * NC_).astype(np.int64)
        es = src[m].astype(np.int64)
        half = (es % NC_) >= A_LOC           # False: A-sourced, True: B
        w = ed // WIN
        cntA[c] = np.bincount(w[~half], minlength=n_win)
        cntB[c] = np.bincount(w[half], minlength=n_win)
        cores.append((ed, es, half, w))

    KA = np.maximum(1, -(-cntA.max(axis=0) // P)).astype(np.int64)
    KB = np.maximum(1, -(-cntB.max(axis=0) // P)).astype(np.int64)
    nA = int(KA.sum())
    tot_chunks = nA + int(KB.sum())
    tot_slots = tot_chunks * P
    # chunk index of each cell: A cells first (by w), then B cells (by w)
    cA0 = np.concatenate([[0], np.cumsum(KA)])[:-1]
    cB0 = nA + np.concatenate([[0], np.cumsum(KB)])[:-1]

    per_core = []
    for c in range(C):
        ed, es, half, w = cores[c]
        slot_src = np.zeros(tot_slots, np.int64)
        slot_dstrel = np.full(tot_slots, -16000.0, np.float32)
        slot_invd = np.zeros(tot_slots, np.float32)
        for wi in range(n_win):
            for hb, K0, cnt in ((False, cA0, cntA), (True, cB0, cntB)):
                sel = (w == wi) & (half == hb)
                n = int(cnt[c, wi])
                s0 = int(K0[wi]) * P
                slot_src[s0:s0 + n] = es[sel]
                slot_dstrel[s0:s0 + n] = (ed[sel] - wi * WIN).astype(np.float32)
                slot_invd[s0:s0 + n] = invdeg[ed[sel] + c * NC_]
        per_core.append((slot_src, slot_dstrel, slot_invd))

    return {
        'NC_': NC_, 'n_win': n_win, 'NPAD': NPAD, 'a_win': a_win,
        'A_LOC': A_LOC, 'B_LOC': NPAD - A_LOC,
        'KA': KA, 'KB': KB, 'cA0': cA0, 'cB0': cB0,
        'tot_chunks': tot_chunks, 'tot_slots': tot_slots,
        'per_core': per_core,
    }


def _build_graph(S, n_nodes):
    NC_, n_win, NPAD = S['NC_'], S['n_win'], S['NPAD']
    a_win, A_LOC, B_LOC = S['a_win'], S['A_LOC'], S['B_LOC']
    KA, KB, cA0, cB0 = S['KA'], S['KB'], S['cA0'], S['cB0']
    tot_chunks = S['tot_chunks']
    Kmax = int(max(KA.max(), KB.max()))
    SB = WIN // P                       # sub-blocks per window

    nc = bacc.Bacc("TRN2", target_bir_lowering=False, debug=False,
                   num_devices=C)

    g1_d = nc.dram_tensor("g1", [P, tot_chunks * F], dt.bfloat16, kind="ExternalInput")
    g2i_d = nc.dram_tensor("g2i", [P, tot_chunks], dt.int32, kind="ExternalInput")
    dstrel_d = nc.dram_tensor("dstrel", [P, tot_chunks], dt.float32, kind="ExternalInput")
    invd_d = nc.dram_tensor("invd", [P, tot_chunks], dt.float32, kind="ExternalInput")
    xownT_d = nc.dram_tensor("xownT", [F, NPAD], dt.bfloat16, kind="ExternalInput")
    iota_d = nc.dram_tensor("iota", [P, WIN], dt.bfloat16, kind="ExternalInput")
    wts_d = nc.dram_tensor("wts", [6, F, F], dt.bfloat16, kind="ExternalInput")
    rows_d = nc.dram_tensor("rows", [4, F], dt.float32, kind="ExternalInput")
    out_d = nc.dram_tensor("out", [2, NPAD, F], dt.float32, kind="ExternalOutput")

    h1shA = nc.dram_tensor("h1shA", [A_LOC, F], dt.float8e4)
    h1shB = nc.dram_tensor("h1shB", [B_LOC, F], dt.float8e4)
    h1fullA = nc.dram_tensor("h1fullA", [C * A_LOC, F], dt.float8e4, addr_space="Shared")
    h1fullB = nc.dram_tensor("h1fullB", [C * B_LOC, F], dt.float8e4, addr_space="Shared")

    with tile.TileContext(nc) as tc:
        with tc.tile_pool(name="cst", bufs=1) as cst, \
             tc.tile_pool(name="gw", bufs=3) as gw, \
             tc.tile_pool(name="g2", bufs=3) as g2p, \
             tc.tile_pool(name="oh", bufs=4) as ohp, \
             tc.tile_pool(name="agg", bufs=2) as aggp, \
             tc.tile_pool(name="st", bufs=3) as stp, \
             tc.tile_pool(name="res", bufs=1) as resp, \
             tc.tile_pool(name="psA", bufs=2, space="PSUM") as psA, \
             tc.tile_pool(name="psA2", bufs=2, space="PSUM") as psA2, \
             tc.tile_pool(name="psH", bufs=2, space="PSUM") as psH, \
             tc.tile_pool(name="psT", bufs=1, space="PSUM") as psT, \
             tc.tile_pool(name="psD", bufs=1, space="PSUM") as psD:

            # ---- constants / tables ----
            iota_t = cst.tile([P, WIN], dt.bfloat16)
            nc.sync.dma_start(iota_t[:], iota_d[:])
            warm_ps = psH.tile([P, F], dt.float32, tag="ps", name="warm_ps")
            for _ in range(40):
                nc.tensor.matmul(out=warm_ps[:], lhsT=iota_t[:, :P],
                                 rhs=iota_t[:, :P], start=True, stop=True)
            w_t = [cst.tile([F, F], dt.bfloat16, tag=f"w{i}", name=f"w{i}") for i in range(6)]
            for i in range(6):
                nc.sync.dma_start(w_t[i][:], wts_d[i])
            r_t = [cst.tile([P, F], dt.float32, tag=f"r{i}", name=f"r{i}") for i in range(4)]
            for i in range(4):
                nc.sync.dma_start(r_t[i][:1, :], rows_d[i][None, :])
            ones_t, b1_t, b2_t, bd_t = r_t
            ident_t = cst.tile([P, P], dt.bfloat16)
            make_identity(nc, ident_t[:])

            dstrel_t = cst.tile([P, tot_chunks], dt.float32)
            invd_t = cst.tile([P, tot_chunks], dt.float32)
            g2i_t = cst.tile([P, tot_chunks], dt.int32)
            nc.sync.dma_start(dstrel_t[:], dstrel_d[:])
            nc.sync.dma_start(invd_t[:], invd_d[:])
            nc.sync.dma_start(g2i_t[:], g2i_d[:])
            xownT_t = cst.tile([F, NPAD], dt.bfloat16)
            nc.sync.dma_start(xownT_t[:], xownT_d[:])
            h1relu_t = resp.tile([P, NPAD], dt.bfloat16)
            aggA_t = resp.tile([F, n_win * WIN], dt.bfloat16)   # L2 A-half partials

            Copy = mybir.ActivationFunctionType.Copy
            Relu = mybir.ActivationFunctionType.Relu

            def onehot(c0, k, scale):
                oh = ohp.tile([P, WIN], dt.bfloat16, name="oh")
                kw = dict(op1=mybir.AluOpType.mult) if scale else {}
                nc.vector.tensor_scalar(
                    out=oh[:], in0=iota_t[:],
                    scalar1=dstrel_t[:, c0 + k:c0 + k + 1],
                    scalar2=invd_t[:, c0 + k:c0 + k + 1] if scale else None,
                    op0=mybir.AluOpType.is_equal, **kw)
                return oh

            def dense_block(aggT_sb, ownT_ap, wl, wr, brow_t, psum_pool):
                ps = psum_pool.tile([P, F], dt.float32, name="ps")
                nc.tensor.matmul(out=ps[:], lhsT=aggT_sb, rhs=wl[:], start=True, stop=False)
                nc.tensor.matmul(out=ps[:], lhsT=ownT_ap, rhs=wr[:], start=False, stop=False)
                nc.tensor.matmul(out=ps[:], lhsT=ones_t[:1, :], rhs=brow_t[:1, :], start=False, stop=True)
                return ps

            # ---------------- layer 1 ----------------
            def l1_window(w):
                psa = psA.tile([F, WIN], dt.float32, name="psa")
                first = True
                for K0, KX in ((cA0, KA), (cB0, KB)):
                    kk, c0 = int(KX[w]), int(K0[w])
                    gt = gw.tile([P, Kmax * F], dt.bfloat16, tag="g1w", name="g1w")
                    nc.sync.dma_start(gt[:, :kk * F], g1_d[:, c0 * F:(c0 + kk) * F])
                    for k in range(kk):
                        oh = onehot(c0, k, False)
                        last = (K0 is cB0) and (k == kk - 1)
                        nc.tensor.matmul(out=psa[:], lhsT=gt[:, k * F:(k + 1) * F],
                                         rhs=oh[:], start=first, stop=last)
                        first = False
                aggT = aggp.tile([F, WIN], dt.bfloat16, tag="aggT", name="aggT")
                nc.scalar.activation(aggT[:], psa[:], Copy)
                for sb in range(SB):
                    blk = w * SB + sb
                    ps = dense_block(aggT[:, sb * P:(sb + 1) * P],
                                     xownT_t[:, blk * P:(blk + 1) * P],
                                     w_t[0], w_t[1], b1_t, psH)
                    nc.scalar.activation(h1relu_t[:, blk * P:(blk + 1) * P], ps[:], Relu)
                    h8 = stp.tile([P, F], dt.float8e4, tag="h8", name="h8")
                    nc.vector.tensor_copy(h8[:], h1relu_t[:, blk * P:(blk + 1) * P])
                    if blk * P < A_LOC:
                        nc.sync.dma_start(h1shA[blk * P:(blk + 1) * P, :], h8[:])
                    else:
                        nc.sync.dma_start(h1shB[blk * P - A_LOC:(blk + 1) * P - A_LOC, :], h8[:])

            for w in range(a_win):
                l1_window(w)
            nc.gpsimd.collective_compute(
                "AllGather", mybir.AluOpType.bypass,
                ins=[h1shA[:]], outs=[h1fullA[:]],
                replica_groups=[list(range(C))])

            # ---------------- layer 2 pass A (overlaps L1 tail) ----------------
            def l2_cells(w, table, K0, KX, psum_pool, inject_A):
                kk, c0 = int(KX[w]), int(K0[w])
                gt = g2p.tile([P, Kmax * F], dt.float8e4, tag="g2w", name="g2w")
                gb = g2p.tile([P, Kmax * F], dt.bfloat16, tag="g2b", name="g2b")
                for k in range(kk):
                    nc.gpsimd.indirect_dma_start(
                        out=gt[:, k * F:(k + 1) * F], out_offset=None,
                        in_=table[:, :],
                        in_offset=bass.IndirectOffsetOnAxis(
                            ap=g2i_t[:, c0 + k:c0 + k + 1], axis=0))
                    nc.vector.tensor_scalar_mul(
                        gb[:, k * F:(k + 1) * F], gt[:, k * F:(k + 1) * F],
                        invd_t[:, c0 + k:c0 + k + 1])
                psa = psum_pool.tile([F, WIN], dt.float32, name="psa2")
                if inject_A:
                    nc.tensor.matmul(out=psa[:], lhsT=ident_t[:],
                                     rhs=aggA_t[:, w * WIN:(w + 1) * WIN],
                                     start=True, stop=False)
                for k in range(kk):
                    oh = onehot(c0, k, False)
                    nc.tensor.matmul(out=psa[:], lhsT=gb[:, k * F:(k + 1) * F],
                                     rhs=oh[:], start=(k == 0 and not inject_A),
                                     stop=(k == kk - 1))
                return psa

            def l2a_window(w):
                psa = l2_cells(w, h1fullA, cA0, KA, psA2, False)
                nc.scalar.activation(aggA_t[:, w * WIN:(w + 1) * WIN], psa[:], Copy)

            # interleave remaining L1 windows with layer-2 pass-A windows so
            # each engine's FIFO queue alternates between the two phases
            seqB = list(range(a_win, n_win))
            seqA = list(range(n_win))
            nb, na = len(seqB), len(seqA)
            ia = 0
            for i, w in enumerate(seqB):
                l1_window(w)
                want = (i + 1) * na // nb
                while ia < min(want, na):
                    l2a_window(seqA[ia]); ia += 1
            while ia < na:
                l2a_window(seqA[ia]); ia += 1
            nc.gpsimd.collective_compute(
                "AllGather", mybir.AluOpType.bypass,
                ins=[h1shB[:]], outs=[h1fullB[:]],
                replica_groups=[list(range(C))])

            # ---------------- layer 2 pass B + epilogues ----------------
            for w in range(n_win):
                psa = l2_cells(w, h1fullB, cB0, KB, psA2, True)
                aggT = aggp.tile([F, WIN], dt.bfloat16, tag="aggT", name="aggT2")
                nc.scalar.activation(aggT[:], psa[:], Copy)
                for sb in range(SB):
                    blk = w * SB + sb
                    pst = psT.tile([P, P], dt.bfloat16, name="pst")
                    nc.tensor.transpose(out=pst[:], in_=h1relu_t[:, blk * P:(blk + 1) * P],
                                        identity=ident_t[:])
                    h1rT = stp.tile([P, P], dt.bfloat16, tag="h1rT", name="h1rT")
                    nc.scalar.activation(h1rT[:], pst[:], Copy)
                    aggT_sb = aggT[:, sb * P:(sb + 1) * P]
                    ps2 = dense_block(aggT_sb, h1rT[:], w_t[2], w_t[3], b2_t, psH)
                    h2sb = stp.tile([P, F], dt.float32, tag="h2sb", name="h2sb")
                    nc.scalar.activation(h2sb[:], ps2[:], Copy)
                    nc.sync.dma_start(out_d[0, blk * P:(blk + 1) * P, :], h2sb[:])
                    psd = dense_block(aggT_sb, h1rT[:], w_t[4], w_t[5], bd_t, psD)
                    dxsb = stp.tile([P, F], dt.float32, tag="dxsb", name="dxsb")
                    nc.scalar.activation(dxsb[:], psd[:], Copy)
                    nc.sync.dma_start(out_d[1, blk * P:(blk + 1) * P, :], dxsb[:])

    nc.compile()
    return nc


def _prep(x, xedge, w1_l, b1_l, w1_r, w2_l, b2_l, w2_r, w_dec, b_dec):
    x = np.asarray(x, dtype=np.float32)
    xedge = np.asarray(xedge)
    n_nodes = x.shape[0]
    src, dst = xedge[0].astype(np.int64), xedge[1].astype(np.int64)
    S = _schedule(src, dst, n_nodes)
    NC_, NPAD = S['NC_'], S['NPAD']
    A_LOC, B_LOC = S['A_LOC'], S['B_LOC']
    tot_chunks = S['tot_chunks']

    xb = x.astype(bf16)
    w1_l = np.asarray(w1_l, np.float32); w1_r = np.asarray(w1_r, np.float32)
    w2_l = np.asarray(w2_l, np.float32); w2_r = np.asarray(w2_r, np.float32)
    w_dec = np.asarray(w_dec, np.float32)
    b1_l = np.asarray(b1_l, np.float32); b2_l = np.asarray(b2_l, np.float32)
    b_dec = np.asarray(b_dec, np.float32)
    wts = np.stack([
        w1_l.T, w1_r.T, w2_l.T, w2_r.T,
        (w_dec @ w2_l).T, (w_dec @ w2_r).T,
    ]).astype(bf16)
    rows = np.stack([
        np.ones(F, np.float32), b1_l, b2_l, (b2_l @ w_dec.T + b_dec),
    ]).astype(np.float32)
    iota = np.tile(np.arange(WIN, dtype=np.float32)[None, :], (P, 1)).astype(bf16)

    in_maps = []
    for c in range(C):
        slot_src, slot_dstrel, slot_invd = S['per_core'][c]
        g1 = np.ascontiguousarray(
            (xb[slot_src].astype(np.float32) * slot_invd[:, None]).astype(bf16)
            .reshape(tot_chunks, P, F).transpose(1, 0, 2)
        ).reshape(P, tot_chunks * F)
        owner = slot_src // NC_
        loc = slot_src % NC_
        gpid = np.where(loc < A_LOC,
                        owner * A_LOC + loc,
                        owner * B_LOC + (loc - A_LOC))
        g2i = gpid.reshape(tot_chunks, P).T.astype(np.int32).copy()
        dstrel = slot_dstrel.reshape(tot_chunks, P).T.copy()
        invd = slot_invd.reshape(tot_chunks, P).T.copy()
        xown = np.zeros((NPAD, F), np.float32)
        xown[:NC_] = x[c * NC_:(c + 1) * NC_]
        in_maps.append({
            "g1": g1,
            "g2i": g2i, "dstrel": dstrel, "invd": invd,
            "xownT": np.ascontiguousarray(xown.T.astype(bf16)),
            "iota": np.asarray(iota), "wts": wts, "rows": rows,
        })

    return S, in_maps


def kernel(x, xedge, w1_l, b1_l, w1_r, w2_l, b2_l, w2_r, w_dec, b_dec):
    x = np.asarray(x, dtype=np.float32)
    xedge = np.asarray(xedge)
    n_nodes = x.shape[0]
    srchead = np.asarray(xedge[0][:16]).astype(np.int64)
    cache_key = (n_nodes, xedge.shape[1], int(srchead.sum()))
    S, in_maps = _prep(x, xedge, w1_l, b1_l, w1_r, w2_l, b2_l, w2_r, w_dec, b_dec)
    NC_ = S['NC_']
    if getattr(kernel, "_cache", None) and kernel._cache[0] == cache_key:
        nc = kernel._cache[1]
    else:
        nc = _build_graph(S, n_nodes)
        kernel._cache = (cache_key, nc)

    from concourse.bass_utils import run_bass_kernel_spmd
    trace = os.environ.get("GSAGE_TRACE", "0") == "1"
    if trace:
        sys.path.insert(0, os.path.dirname(os.path.abspath(__file__)))
        import axprof  # noqa: F401
    res = run_bass_kernel_spmd(nc, in_maps, core_ids=list(range(C)), trace=trace)
    if trace:
        kernel.last_exec_time_ns = res.exec_time_ns

    h = np.empty((n_nodes, F), np.float32)
    dx = np.empty((n_nodes, F), np.float32)
    for c in range(C):
        o = res.results[c]["out"]
        h[c * NC_:(c + 1) * NC_] = o[0, :NC_]
        dx[c * NC_:(c + 1) * NC_] = o[1, :NC_]
    return (h, dx)


# revision 16
# speedup vs baseline: 1.0981x; 1.0981x over previous
"""GraphSAGE (2-layer + decoder) on 8 TRN2 NeuronCores.

Sharding: nodes partitioned across 8 cores (dst-partitioned edges).
Layer 1 feeds on a host-gathered, edge-ordered copy of x (sequential DMA);
relu(h1) shards are AllGathered in two pieces (bf16), and layer 2 gathers
source rows on-device with indirect DMA. Segment-mean is computed as one-hot
matmuls accumulating into per-window PSUM; dense lin_l/lin_r/bias terms and
the decoder are fused per 128-node block. Decoder weights are pre-composed
with conv2 weights on the host so dx needs no extra transpose.

Pipelining: each core's nodes are split into an early slice A (first A_WIN
windows) and the rest B. Sequence: L1(A) -> AllGather(A) -> L1(B) with
layer-2 gathers/aggregation for A-sourced edges overlapping it -> AG(B) ->
layer-2 B-sourced aggregation (seeded with the buffered A partials) +
epilogues. This keeps the serialized indirect-DMA stream (the bottleneck)
running as early as possible.
"""
import os
import sys

sys.path.insert(0, '/opt/trn_rl_repo')

import numpy as np
import ml_dtypes

import concourse.bass as bass
import concourse.bacc as bacc
import concourse.mybir as mybir
import concourse.tile as tile
from concourse.masks import make_identity

bf16 = ml_dtypes.bfloat16
dt = mybir.dt

C = 8           # cores
F = 128         # features/hidden
WIN = 256       # dst nodes per aggregation window (one-hot width)
P = 128         # partitions / chunk size


def _schedule(src, dst, n_nodes):
    """Shared (SPMD-uniform) schedule with (window, source-half) cells."""
    NC_ = n_nodes // C
    n_win = (NC_ + WIN - 1) // WIN
    NPAD = n_win * WIN
    a_win = max(1, n_win // 5)
    A_LOC = a_win * WIN                      # early-slice nodes per core
    deg = np.bincount(dst, minlength=n_nodes)
    invdeg = (1.0 / np.maximum(deg, 1)).astype(np.float32)

    cores = []
    cntA = np.zeros((C, n_win), np.int64)
    cntB = np.zeros((C, n_win), np.int64)
    for c in range(C):
        m = (dst >= c * NC_) & (dst < (c + 1) * NC_)
        ed = (dst[m] - c * NC_).astype(np.int64)
        es = src[m].astype(np.int64)
        half = (es % NC_) >= A_LOC           # False: A-sourced, True: B
        w = ed // WIN
        cntA[c] = np.bincount(w[~half], minlength=n_win)
        cntB[c] = np.bincount(w[half], minlength=n_win)
        cores.append((ed, es, half, w))

    KA = np.maximum(1, -(-cntA.max(axis=0) // P)).astype(np.int64)
    KB = np.maximum(1, -(-cntB.max(axis=0) // P)).astype(np.int64)
    nA = int(KA.sum())
    tot_chunks = nA + int(KB.sum())
    tot_slots = tot_chunks * P
    # chunk index of each cell: A cells first (by w), then B cells (by w)
    cA0 = np.concatenate([[0], np.cumsum(KA)])[:-1]
    cB0 = nA + np.concatenate([[0], np.cumsum(KB)])[:-1]

    per_core = []
    for c in range(C):
        ed, es, half, w = cores[c]
        slot_src = np.zeros(tot_slots, np.int64)
        slot_dstrel = np.full(tot_slots, -16000.0, np.float32)
        slot_invd = np.zeros(tot_slots, np.float32)
        for wi in range(n_win):
            for hb, K0, cnt in ((False, cA0, cntA), (True, cB0, cntB)):
                sel = (w == wi) & (half == hb)
                n = int(cnt[c, wi])
                s0 = int(K0[wi]) * P
                slot_src[s0:s0 + n] = es[sel]
                slot_dstrel[s0:s0 + n] = (ed[sel] - wi * WIN).astype(np.float32)
                slot_invd[s0:s0 + n] = invdeg[ed[sel] + c * NC_]
        per_core.append((slot_src, slot_dstrel, slot_invd))

    return {
        'NC_': NC_, 'n_win': n_win, 'NPAD': NPAD, 'a_win': a_win,
        'A_LOC': A_LOC, 'B_LOC': NPAD - A_LOC,
        'KA': KA, 'KB': KB, 'cA0': cA0, 'cB0': cB0,
        'tot_chunks': tot_chunks, 'tot_slots': tot_slots,
        'per_core': per_core,
    }


def _build_graph(S, n_nodes):
    NC_, n_win, NPAD = S['NC_'], S['n_win'], S['NPAD']
    a_win, A_LOC, B_LOC = S['a_win'], S['A_LOC'], S['B_LOC']
    KA, KB, cA0, cB0 = S['KA'], S['KB'], S['cA0'], S['cB0']
    tot_chunks = S['tot_chunks']
    Kmax = int(max(KA.max(), KB.max()))
    SB = WIN // P                       # sub-blocks per window

    nc = bacc.Bacc("TRN2", target_bir_lowering=False, debug=False,
                   num_devices=C)

    g1_d = nc.dram_tensor("g1", [P, tot_chunks * F], dt.bfloat16, kind="ExternalInput")
    g2i_d = nc.dram_tensor("g2i", [P, tot_chunks], dt.int32, kind="ExternalInput")
    dstrel_d = nc.dram_tensor("dstrel", [P, tot_chunks], dt.float32, kind="ExternalInput")
    invd_d = nc.dram_tensor("invd", [P, tot_chunks], dt.float32, kind="ExternalInput")
    xownT_d = nc.dram_tensor("xownT", [F, NPAD], dt.bfloat16, kind="ExternalInput")
    iota_d = nc.dram_tensor("iota", [P, WIN], dt.bfloat16, kind="ExternalInput")
    wts_d = nc.dram_tensor("wts", [6, F, F], dt.bfloat16, kind="ExternalInput")
    rows_d = nc.dram_tensor("rows", [4, F], dt.float32, kind="ExternalInput")
    out_d = nc.dram_tensor("out", [2, NPAD, F], dt.float32, kind="ExternalOutput")

    h1shA = nc.dram_tensor("h1shA", [A_LOC, F], dt.bfloat16)
    h1shB = nc.dram_tensor("h1shB", [B_LOC, F], dt.bfloat16)
    h1fullA = nc.dram_tensor("h1fullA", [C * A_LOC, F], dt.bfloat16, addr_space="Shared")
    h1fullB = nc.dram_tensor("h1fullB", [C * B_LOC, F], dt.bfloat16, addr_space="Shared")

    with tile.TileContext(nc) as tc:
        with tc.tile_pool(name="cst", bufs=1) as cst, \
             tc.tile_pool(name="gw", bufs=3) as gw, \
             tc.tile_pool(name="g2", bufs=3) as g2p, \
             tc.tile_pool(name="oh", bufs=4) as ohp, \
             tc.tile_pool(name="agg", bufs=2) as aggp, \
             tc.tile_pool(name="st", bufs=3) as stp, \
             tc.tile_pool(name="res", bufs=1) as resp, \
             tc.tile_pool(name="psA", bufs=2, space="PSUM") as psA, \
             tc.tile_pool(name="psA2", bufs=2, space="PSUM") as psA2, \
             tc.tile_pool(name="psH", bufs=2, space="PSUM") as psH, \
             tc.tile_pool(name="psT", bufs=1, space="PSUM") as psT, \
             tc.tile_pool(name="psD", bufs=1, space="PSUM") as psD:

            # ---- constants / tables ----
            iota_t = cst.tile([P, WIN], dt.bfloat16)
            nc.sync.dma_start(iota_t[:], iota_d[:])
            warm_ps = psH.tile([P, F], dt.float32, tag="ps", name="warm_ps")
            for _ in range(40):
                nc.tensor.matmul(out=warm_ps[:], lhsT=iota_t[:, :P],
                                 rhs=iota_t[:, :P], start=True, stop=True)
            w_t = [cst.tile([F, F], dt.bfloat16, tag=f"w{i}", name=f"w{i}") for i in range(6)]
            for i in range(6):
                nc.sync.dma_start(w_t[i][:], wts_d[i])
            r_t = [cst.tile([P, F], dt.float32, tag=f"r{i}", name=f"r{i}") for i in range(4)]
            for i in range(4):
                nc.sync.dma_start(r_t[i][:1, :], rows_d[i][None, :])
            ones_t, b1_t, b2_t, bd_t = r_t
            ident_t = cst.tile([P, P], dt.bfloat16)
            make_identity(nc, ident_t[:])

            dstrel_t = cst.tile([P, tot_chunks], dt.float32)
            invd_t = cst.tile([P, tot_chunks], dt.float32)
            g2i_t = cst.tile([P, tot_chunks], dt.int32)
            nc.sync.dma_start(dstrel_t[:], dstrel_d[:])
            nc.sync.dma_start(invd_t[:], invd_d[:])
            nc.sync.dma_start(g2i_t[:], g2i_d[:])
            xownT_t = cst.tile([F, NPAD], dt.bfloat16)
            nc.sync.dma_start(xownT_t[:], xownT_d[:])
            h1relu_t = resp.tile([P, NPAD], dt.bfloat16)
            aggA_t = resp.tile([F, n_win * WIN], dt.bfloat16)   # L2 A-half partials

            Copy = mybir.ActivationFunctionType.Copy
            Relu = mybir.ActivationFunctionType.Relu

            def onehot(c0, k, scale):
                oh = ohp.tile([P, WIN], dt.bfloat16, name="oh")
                kw = dict(op1=mybir.AluOpType.mult) if scale else {}
                nc.vector.tensor_scalar(
                    out=oh[:], in0=iota_t[:],
                    scalar1=dstrel_t[:, c0 + k:c0 + k + 1],
                    scalar2=invd_t[:, c0 + k:c0 + k + 1] if scale else None,
                    op0=mybir.AluOpType.is_equal, **kw)
                return oh

            def dense_block(aggT_sb, ownT_ap, wl, wr, brow_t, psum_pool):
                ps = psum_pool.tile([P, F], dt.float32, name="ps")
                nc.tensor.matmul(out=ps[:], lhsT=aggT_sb, rhs=wl[:], start=True, stop=False)
                nc.tensor.matmul(out=ps[:], lhsT=ownT_ap, rhs=wr[:], start=False, stop=False)
                nc.tensor.matmul(out=ps[:], lhsT=ones_t[:1, :], rhs=brow_t[:1, :], start=False, stop=True)
                return ps

            # ---------------- layer 1 ----------------
            def l1_window(w):
                psa = psA.tile([F, WIN], dt.float32, name="psa")
                first = True
                for K0, KX in ((cA0, KA), (cB0, KB)):
                    kk, c0 = int(KX[w]), int(K0[w])
                    gt = gw.tile([P, Kmax * F], dt.bfloat16, tag="g1w", name="g1w")
                    nc.sync.dma_start(gt[:, :kk * F], g1_d[:, c0 * F:(c0 + kk) * F])
                    for k in range(kk):
                        oh = onehot(c0, k, False)
                        last = (K0 is cB0) and (k == kk - 1)
                        nc.tensor.matmul(out=psa[:], lhsT=gt[:, k * F:(k + 1) * F],
                                         rhs=oh[:], start=first, stop=last)
                        first = False
                aggT = aggp.tile([F, WIN], dt.bfloat16, tag="aggT", name="aggT")
                nc.scalar.activation(aggT[:], psa[:], Copy)
                for sb in range(SB):
                    blk = w * SB + sb
                    ps = dense_block(aggT[:, sb * P:(sb + 1) * P],
                                     xownT_t[:, blk * P:(blk + 1) * P],
                                     w_t[0], w_t[1], b1_t, psH)
                    nc.scalar.activation(h1relu_t[:, blk * P:(blk + 1) * P], ps[:], Relu)
                    if blk * P < A_LOC:
                        nc.sync.dma_start(h1shA[blk * P:(blk + 1) * P, :],
                                          h1relu_t[:, blk * P:(blk + 1) * P])
                    else:
                        nc.sync.dma_start(h1shB[blk * P - A_LOC:(blk + 1) * P - A_LOC, :],
                                          h1relu_t[:, blk * P:(blk + 1) * P])

            for w in range(a_win):
                l1_window(w)
            nc.gpsimd.collective_compute(
                "AllGather", mybir.AluOpType.bypass,
                ins=[h1shA[:]], outs=[h1fullA[:]],
                replica_groups=[list(range(C))])

            # ---------------- layer 2 pass A (overlaps L1 tail) ----------------
            def l2_cells(w, table, K0, KX, psum_pool, inject_A):
                kk, c0 = int(KX[w]), int(K0[w])
                gt = g2p.tile([P, Kmax * F], dt.bfloat16, tag="g2w", name="g2w")
                gb = gt
                for k in range(kk):
                    nc.gpsimd.indirect_dma_start(
                        out=gt[:, k * F:(k + 1) * F], out_offset=None,
                        in_=table[:, :],
                        in_offset=bass.IndirectOffsetOnAxis(
                            ap=g2i_t[:, c0 + k:c0 + k + 1], axis=0))
                psa = psum_pool.tile([F, WIN], dt.float32, name="psa2")
                if inject_A:
                    nc.tensor.matmul(out=psa[:], lhsT=ident_t[:],
                                     rhs=aggA_t[:, w * WIN:(w + 1) * WIN],
                                     start=True, stop=False)
                for k in range(kk):
                    oh = onehot(c0, k, True)
                    nc.tensor.matmul(out=psa[:], lhsT=gb[:, k * F:(k + 1) * F],
                                     rhs=oh[:], start=(k == 0 and not inject_A),
                                     stop=(k == kk - 1))
                return psa

            def l2a_window(w):
                psa = l2_cells(w, h1fullA, cA0, KA, psA2, False)
                nc.scalar.activation(aggA_t[:, w * WIN:(w + 1) * WIN], psa[:], Copy)

            # interleave remaining L1 windows with layer-2 pass-A windows so
            # each engine's FIFO queue alternates between the two phases
            seqB = list(range(a_win, n_win))
            seqA = list(range(n_win))
            nb, na = len(seqB), len(seqA)
            ia = 0
            for i, w in enumerate(seqB):
                l1_window(w)
                want = (i + 1) * na // nb
                while ia < min(want, na):
                    l2a_window(seqA[ia]); ia += 1
            while ia < na:
                l2a_window(seqA[ia]); ia += 1
            nc.gpsimd.collective_compute(
                "AllGather", mybir.AluOpType.bypass,
                ins=[h1shB[:]], outs=[h1fullB[:]],
                replica_groups=[list(range(C))])

            # ---------------- layer 2 pass B + epilogues ----------------
            for w in range(n_win):
                psa = l2_cells(w, h1fullB, cB0, KB, psA2, True)
                aggT = aggp.tile([F, WIN], dt.bfloat16, tag="aggT", name="aggT2")
                nc.scalar.activation(aggT[:], psa[:], Copy)
                for sb in range(SB):
                    blk = w * SB + sb
                    pst = psT.tile([P, P], dt.bfloat16, name="pst")
                    nc.tensor.transpose(out=pst[:], in_=h1relu_t[:, blk * P:(blk + 1) * P],
                                        identity=ident_t[:])
                    h1rT = stp.tile([P, P], dt.bfloat16, tag="h1rT", name="h1rT")
                    nc.scalar.activation(h1rT[:], pst[:], Copy)
                    aggT_sb = aggT[:, sb * P:(sb + 1) * P]
                    ps2 = dense_block(aggT_sb, h1rT[:], w_t[2], w_t[3], b2_t, psH)
                    h2sb = stp.tile([P, F], dt.float32, tag="h2sb", name="h2sb")
                    nc.scalar.activation(h2sb[:], ps2[:], Copy)
                    nc.sync.dma_start(out_d[0, blk * P:(blk + 1) * P, :], h2sb[:])
                    psd = dense_block(aggT_sb, h1rT[:], w_t[4], w_t[5], bd_t, psD)
                    dxsb = stp.tile([P, F], dt.float32, tag="dxsb", name="dxsb")
                    nc.scalar.activation(dxsb[:], psd[:], Copy)
                    nc.sync.dma_start(out_d[1, blk * P:(blk + 1) * P, :], dxsb[:])

    nc.compile()
    return nc


def _prep(x, xedge, w1_l, b1_l, w1_r, w2_l, b2_l, w2_r, w_dec, b_dec):
    x = np.asarray(x, dtype=np.float32)
    xedge = np.asarray(xedge)
    n_nodes = x.shape[0]
    src, dst = xedge[0].astype(np.int64), xedge[1].astype(np.int64)
    S = _schedule(src, dst, n_nodes)
    NC_, NPAD = S['NC_'], S['NPAD']
    A_LOC, B_LOC = S['A_LOC'], S['B_LOC']
    tot_chunks = S['tot_chunks']

    xb = x.astype(bf16)
    w1_l = np.asarray(w1_l, np.float32); w1_r = np.asarray(w1_r, np.float32)
    w2_l = np.asarray(w2_l, np.float32); w2_r = np.asarray(w2_r, np.float32)
    w_dec = np.asarray(w_dec, np.float32)
    b1_l = np.asarray(b1_l, np.float32); b2_l = np.asarray(b2_l, np.float32)
    b_dec = np.asarray(b_dec, np.float32)
    wts = np.stack([
        w1_l.T, w1_r.T, w2_l.T, w2_r.T,
        (w_dec @ w2_l).T, (w_dec @ w2_r).T,
    ]).astype(bf16)
    rows = np.stack([
        np.ones(F, np.float32), b1_l, b2_l, (b2_l @ w_dec.T + b_dec),
    ]).astype(np.float32)
    iota = np.tile(np.arange(WIN, dtype=np.float32)[None, :], (P, 1)).astype(bf16)

    in_maps = []
    for c in range(C):
        slot_src, slot_dstrel, slot_invd = S['per_core'][c]
        g1 = np.ascontiguousarray(
            (xb[slot_src].astype(np.float32) * slot_invd[:, None]).astype(bf16)
            .reshape(tot_chunks, P, F).transpose(1, 0, 2)
        ).reshape(P, tot_chunks * F)
        owner = slot_src // NC_
        loc = slot_src % NC_
        gpid = np.where(loc < A_LOC,
                        owner * A_LOC + loc,
                        owner * B_LOC + (loc - A_LOC))
        g2i = gpid.reshape(tot_chunks, P).T.astype(np.int32).copy()
        dstrel = slot_dstrel.reshape(tot_chunks, P).T.copy()
        invd = slot_invd.reshape(tot_chunks, P).T.copy()
        xown = np.zeros((NPAD, F), np.float32)
        xown[:NC_] = x[c * NC_:(c + 1) * NC_]
        in_maps.append({
            "g1": g1,
            "g2i": g2i, "dstrel": dstrel, "invd": invd,
            "xownT": np.ascontiguousarray(xown.T.astype(bf16)),
            "iota": np.asarray(iota), "wts": wts, "rows": rows,
        })

    return S, in_maps


def kernel(x, xedge, w1_l, b1_l, w1_r, w2_l, b2_l, w2_r, w_dec, b_dec):
    x = np.asarray(x, dtype=np.float32)
    xedge = np.asarray(xedge)
    n_nodes = x.shape[0]
    srchead = np.asarray(xedge[0][:16]).astype(np.int64)
    cache_key = (n_nodes, xedge.shape[1], int(srchead.sum()))
    S, in_maps = _prep(x, xedge, w1_l, b1_l, w1_r, w2_l, b2_l, w2_r, w_dec, b_dec)
    NC_ = S['NC_']
    if getattr(kernel, "_cache", None) and kernel._cache[0] == cache_key:
        nc = kernel._cache[1]
    else:
        nc = _build_graph(S, n_nodes)
        kernel._cache = (cache_key, nc)

    from concourse.bass_utils import run_bass_kernel_spmd
    trace = os.environ.get("GSAGE_TRACE", "0") == "1"
    if trace:
        sys.path.insert(0, os.path.dirname(os.path.abspath(__file__)))
        import axprof  # noqa: F401
    res = run_bass_kernel_spmd(nc, in_maps, core_ids=list(range(C)), trace=trace)
    if trace:
        kernel.last_exec_time_ns = res.exec_time_ns

    h = np.empty((n_nodes, F), np.float32)
    dx = np.empty((n_nodes, F), np.float32)
    for c in range(C):
        o = res.results[c]["out"]
        h[c * NC_:(c + 1) * NC_] = o[0, :NC_]
        dx[c * NC_:(c + 1) * NC_] = o[1, :NC_]
    return (h, dx)
